# revision 30
# baseline (speedup 1.0000x reference)
"""MultiHeadCrossAttention kernel for 8 trn2 NeuronCores.

Reference computation (fp32, per batch b):
    q = Q[b] @ W_q.T ; k = K[b] @ W_k.T ; v = V[b] @ W_v.T      (heads on columns)
    per head h: S = (q_h @ k_h.T) / 8 ; E = exp(S); A = E / E.sum(-1)
    out[b] = concat_h(A @ v_h) @ W_o.T ; rows with mask==0 zeroed

Sharding: 8 cores = (batch b in {0,1}) x (head-group hg in {0..3}, 4 heads each).
Each core computes a partial output  out_part[b] = concat(heads hg) @ W_o[:, cols].T
(stored bf16) and the host sums the 4 partials per batch in f32.

Per-core pipeline (all matmul operands bf16, fp32 PSUM accumulation):
  - Projections as in the earlier design: xT inputs [1024(in), 2048(seq)],
    q/k results stored [128, 2, 2048] with head-pair chunks so the two scores
    matmuls of a pair run on disjoint PE row groups concurrently.
  - Attention inner loop (128 iterations of (qp, hp, j)): scores pair ->
    exp -> PV. The exp of each [128, 1024] scores tile is SPLIT between the
    Scalar engine (exact Exp, columns 0:512 = even head) and the Vector
    engine (Schraudolph bit-trick exp -> uint16 viewed as bf16, columns
    512:1024 = odd head). This halves the per-iteration exp latency and
    roughly doubles exp throughput, which was the baseline bottleneck.
  - PV accumulators acc_h [128, 512] (row 0 = denominator via the ones row
    of vaug; rows 64:128 = out^T). After j=15 the acc is copied to SBUF by
    ScalarE (releases the PSUM bank fast), the denominator row is broadcast
    across partitions via a DRAM bounce, and VectorE computes
    outT = acc / denom with a single tensor_tensor divide (bf16 out).
  - W_o: final[q,:] accumulated over the 256 local dims; the mask (per-q
    0/1) is applied as the per-partition scale of the PSUM->SBUF copy.
  - PE work is software-pipelined: the PV matmuls of iteration i are
    emitted two iterations later so the PE never waits on the exp engines;
    W_o matmuls/copies and the normalize chain are drained from a pending
    queue at a bounded rate between iterations.
"""

import numpy as np
import ml_dtypes
from collections import deque

import concourse.bass as bass
import concourse.bacc as bacc
import concourse.mybir as mybir
import concourse.tile as tile
from contextlib import ExitStack

F32 = mybir.dt.float32
BF16 = mybir.dt.bfloat16
U16 = mybir.dt.uint16
AF = mybir.ActivationFunctionType
ALU = mybir.AluOpType

B = 2
SEQ = 2048          # Sq == Sk
D = 1024            # model dim
DL = 256            # local head dims per core (4 heads x 64)
HL = 4              # local heads
DH = 64             # head dim
NCORES = 8

# Schraudolph exp in bf16 bit space: bits16 = round(s * 0.125 * 128*log2(e)
# + (127*128 + C)).  C = -7.356 centers the ratio approx/exact at 1.0
# (measured on hw: raw trick has mean ratio 1.04064).
EXPA = 0.125 * 128.0 * 1.4426950408889634
EXPB = 16256.0 - 7.3563

_PROGRAM = None


def build_program():
    nc = bacc.Bacc("TRN2", target_bir_lowering=False)

    xqT = nc.declare_dram_parameter("xqT", [D, SEQ], BF16, isOutput=False)
    xkT = nc.declare_dram_parameter("xkT", [D, SEQ], BF16, isOutput=False)
    xvT = nc.declare_dram_parameter("xvT", [D, SEQ], BF16, isOutput=False)
    wq = nc.declare_dram_parameter("wq", [D, DL], BF16, isOutput=False)
    wk = nc.declare_dram_parameter("wk", [D, DL], BF16, isOutput=False)
    wv = nc.declare_dram_parameter("wv", [D, DL], BF16, isOutput=False)
    wo = nc.declare_dram_parameter("wo", [DL, D], BF16, isOutput=False)
    maskf = nc.declare_dram_parameter("maskf", [128, SEQ // 128], F32, isOutput=False)
    out_part = nc.declare_dram_parameter("out_part", [SEQ, D], BF16, isOutput=True)


    with tile.TileContext(nc) as tc, ExitStack() as ctx:
        const = ctx.enter_context(tc.tile_pool(name="const", bufs=1))
        proj = ctx.enter_context(tc.tile_pool(name="proj", bufs=1))
        xpool = ctx.enter_context(tc.tile_pool(name="xpool", bufs=8))
        epool = ctx.enter_context(tc.tile_pool(name="epool", bufs=5))
        opool = ctx.enter_context(tc.tile_pool(name="opool", bufs=2))
        ospool = ctx.enter_context(tc.tile_pool(name="ospool", bufs=3))
        rpool = ctx.enter_context(tc.tile_pool(name="rpool", bufs=4))
        # PSUM: tag st 3x[128,1024] = 6 banks (scores, proj, and wo tiles all
        # rotate this ring), tag acc 2x[128,512] = 2 banks -> 8 total.
        stp = ctx.enter_context(tc.tile_pool(name="stp", bufs=3, space="PSUM"))
        accp = ctx.enter_context(
            tc.tile_pool(name="accp", bufs=2, space="PSUM")
        )
        wop = stp

        # ---------------- constants + input prologue ----------------
        # Critical path to the first k-proj matmul: wk + xk half0 on the
        # sync queue; xk half1 goes on the scalar queue in parallel. The
        # remaining const DMAs queue behind so they don't delay the start.
        wq_sb = const.tile([128, 8, DL], BF16)
        wk_sb = const.tile([128, 8, DL], BF16)
        wv_sb = const.tile([128, 8, DL], BF16)
        wo_sb = const.tile([128, 2, D], BF16)
        mask_sb = const.tile([128, SEQ // 128], F32)
        ones_sb = const.tile([1, 128], BF16)
        nc.vector.memset(ones_sb[:], 1.0)
        nc.sync.dma_start(wk_sb[:], wk[:].rearrange("(a p) d -> p a d", p=128))

        x_tiles = {}
        for (which, xT), plan in (
            (("k", xkT), ((0, nc.sync), (1, nc.scalar))),
            (("q", xqT), ((0, nc.sync), (1, nc.sync))),
        ):
            for half, eng in plan:
                for quarter in range(2):
                    x_t = xpool.tile(
                        [128, 4, 1024], BF16, tag="x", name=f"x{which}{half}{quarter}"
                    )
                    eng.dma_start(
                        x_t[:],
                        xT[
                            quarter * 512 : (quarter + 1) * 512,
                            half * 1024 : (half + 1) * 1024,
                        ].rearrange("(a p) q -> p a q", p=128),
                    )
                    x_tiles[(which, half, quarter)] = x_t

        nc.scalar.dma_start(wq_sb[:], wq[:].rearrange("(a p) d -> p a d", p=128))
        nc.scalar.dma_start(wv_sb[:], wv[:].rearrange("(a p) d -> p a d", p=128))
        nc.scalar.dma_start(wo_sb[:], wo[:].rearrange("(a p) d -> p a d", p=128))
        nc.scalar.dma_start(mask_sb[:], maskf[:])

        kT0_sb = proj.tile([128, SEQ], BF16)
        kT1_sb = proj.tile([128, SEQ], BF16)
        kTs = (kT0_sb, kT1_sb)
        qTs = [
            [proj.tile([128, 1024], BF16, name=f"qT{dm}_{h}") for h in range(2)]
            for dm in range(2)
        ]
        vaugs = [
            proj.tile([128, HL, 128], BF16, name=f"vaug{j}") for j in range(16)
        ]
        for j in range(16):
            nc.vector.memset(vaugs[j][:], 0.0)
            nc.vector.memset(vaugs[j][:, :, 0:1], 1.0)

        # ---------------- projections ----------------
        for w_sb, xT, which in ((wk_sb, xkT, "k"), (wq_sb, xqT, "q")):
            for half in range(2):
                x_ts = [x_tiles[(which, half, 0)], x_tiles[(which, half, 1)]]
                for dm in range(2):
                    for qc in range(2):
                        ps = stp.tile([128, 512], F32, tag="st", name=f"ps{which}")
                        for ki in range(8):
                            nc.tensor.matmul(
                                ps[:],
                                lhsT=w_sb[:, ki, dm * 128 : (dm + 1) * 128],
                                rhs=x_ts[ki // 4][:, ki % 4, qc * 512 : (qc + 1) * 512],
                                start=(ki == 0),
                                stop=(ki == 7),
                            )
                        col = half * 1024 + qc * 512
                        if which == "k":
                            nc.scalar.copy(out=kTs[dm][:, col : col + 512], in_=ps[:])
                        else:
                            nc.scalar.copy(
                                out=qTs[dm][half][:, qc * 512 : (qc + 1) * 512],
                                in_=ps[:],
                            )
        # v: out v[j*128+p, dl] ; stationary = xvT chunk, moving = wv
        for half in range(2):
            xv_ts = []
            for quarter in range(2):
                x_t = xpool.tile(
                    [128, 4, 1024], BF16, tag="x", name=f"xv{half}{quarter}"
                )
                nc.sync.dma_start(
                    x_t[:],
                    xvT[
                        quarter * 512 : (quarter + 1) * 512,
                        half * 1024 : (half + 1) * 1024,
                    ].rearrange("(a p) q -> p a q", p=128),
                )
                xv_ts.append(x_t)
            for km in range(8):
                ps = stp.tile([128, 512], F32, tag="st", name="psv")
                for ki in range(8):
                    nc.tensor.matmul(
                        ps[:, 0:DL],
                        lhsT=xv_ts[ki // 4][:, ki % 4, km * 128 : (km + 1) * 128],
                        rhs=wv_sb[:, ki, :],
                        start=(ki == 0),
                        stop=(ki == 7),
                    )
                nc.vector.tensor_copy(
                    vaugs[half * 8 + km][:, :, 64 : 64 + DH],
                    ps[:, 0:DL].rearrange("p (h d) -> p h d", h=HL),
                )

        # ---------------- attention + output projection ----------------
        blocks = [(qp, hp) for qp in range(4) for hp in range(2)]
        outTs = {}
        pend = []          # [ready_gi, fn] FIFO
        avq = deque()      # delayed PV emissions (pipeline skew 2)

        def emit_av(entry):
            j, acc0, acc1, e_t, hp_ = entry
            for hi, acc in ((0, acc0), (1, acc1)):
                h = 2 * hp_ + hi
                nc.tensor.matmul(
                    acc[:],
                    lhsT=vaugs[j][:, h, :],
                    rhs=e_t[:, hi * 512 : (hi + 1) * 512],
                    start=(j == 0),
                    stop=(j == 15),
                )

        def make_norm_entries(b, qp, hp, acc0, acc1, outT_sb):
            """Returns [(delay, fn), ...] for the normalize chain of block b.

            recip+copy release the acc bank quickly; the reciprocal row is
            broadcast across partitions with a PE outer product into a
            borrowed st-ring PSUM tile, then one DVE mul (sbuf x psum)."""
            entries = []
            accs = (acc0, acc1)
            acc_sbs = []
            r_sb = rpool.tile([1, 2, 512], BF16, tag="r", name=f"r{b}")
            rb_ps = {}

            def cp(hi):
                acc_sb = rpool.tile(
                    [128, 512], F32, tag="accsb", name=f"accsb{b}_{hi}"
                )
                acc_sbs.append(acc_sb)
                r_f32 = rpool.tile([1, 512], F32, tag="rf", name=f"rf{b}_{hi}")
                nc.vector.reciprocal_approx_fast(out=r_f32[:], in_=accs[hi][0:1, :])
                nc.scalar.copy(
                    out=acc_sb[64:128, :], in_=accs[hi][64:128, :]
                )
                nc.scalar.copy(out=r_sb[:, hi, :], in_=r_f32[:])

            def outer():
                ps = stp.tile([128, 1024], F32, tag="st", name=f"rbps{b}")
                rb_ps[0] = ps
                for hi in range(2):
                    nc.tensor.matmul(
                        ps[:, hi * 512 : (hi + 1) * 512],
                        lhsT=ones_sb[:],
                        rhs=r_sb[:, hi, :],
                        start=True,
                        stop=True,
                    )

            def mul(hi):
                nc.vector.tensor_mul(
                    outT_sb[hi * 64 : (hi + 1) * 64, hp, :],
                    acc_sbs[hi][64 : 64 + DH, :],
                    rb_ps[0][64:128, hi * 512 : (hi + 1) * 512],
                )

            entries.append((2, lambda: cp(0)))
            entries.append((3, lambda: cp(1)))
            entries.append((6, outer))
            entries.append((8, lambda: mul(0)))
            entries.append((9, lambda: mul(1)))
            return entries

        wo_cp_ctr = [0]

        def make_wo_entries(qp, outT_sb):
            entries = []
            for mq in range(4):
                o_sb = ospool.tile([128, 1024], BF16, tag="o", name=f"o{qp}_{mq}")
                qg = qp * 4 + mq
                ps_ref = {}

                def mk_mm(oc, mq=mq, o_sb=o_sb, ps_ref=ps_ref):
                    def f():
                        ps = wop.tile(
                            [128, 512], F32, tag="st", name=f"wops{qp}_{mq}_{oc}"
                        )
                        ps_ref[oc] = ps
                        for kc in range(2):
                            nc.tensor.matmul(
                                ps[:],
                                lhsT=outT_sb[:, kc, mq * 128 : (mq + 1) * 128],
                                rhs=wo_sb[:, kc, oc * 512 : (oc + 1) * 512],
                                start=(kc == 0),
                                stop=(kc == 1),
                            )
                    return f

                def mk_cp(oc, last, mq=mq, o_sb=o_sb, ps_ref=ps_ref, qg=qg):
                    def f():
                        i = wo_cp_ctr[0]
                        wo_cp_ctr[0] += 1
                        if i % 2 == 0:
                            nc.scalar.activation(
                                out=o_sb[:, oc * 512 : (oc + 1) * 512],
                                in_=ps_ref[oc][:],
                                func=AF.Copy,
                                scale=mask_sb[:, qg : qg + 1],
                            )
                        else:
                            nc.vector.tensor_scalar_mul(
                                o_sb[:, oc * 512 : (oc + 1) * 512],
                                ps_ref[oc][:],
                                mask_sb[:, qg : qg + 1],
                            )
                        if last:
                            nc.sync.dma_start(
                                out_part[qg * 128 : (qg + 1) * 128, :], o_sb[:]
                            )
                    return f

                base = 9 + mq * 6
                entries.append((base, mk_mm(0)))
                entries.append((base + 2, lambda f1=mk_cp(0, False), f2=mk_mm(1): (f1(), f2())))
                entries.append((base + 4, mk_cp(1, True)))
            return entries

        gi = 0
        for b, (qp, hp) in enumerate(blocks):
            acc0 = accp.tile([128, 512], F32, tag="acc", name=f"acc{b}_0")
            acc1 = accp.tile([128, 512], F32, tag="acc", name=f"acc{b}_1")
            if qp not in outTs:
                outTs[qp] = opool.tile(
                    [128, 2, 512], BF16, tag="outT", name=f"outT{qp}"
                )
            outT_sb = outTs[qp]
            q0 = qp * 512
            for j in range(16):
                st = stp.tile([128, 1024], F32, tag="st", name=f"st{b}_{j}")
                for hi in range(2):
                    r0 = hi * 64
                    nc.tensor.matmul(
                        st[:, hi * 512 : (hi + 1) * 512],
                        lhsT=kTs[hp][r0 : r0 + 64, j * 128 : (j + 1) * 128],
                        rhs=qTs[hp][q0 // 1024][
                            r0 : r0 + 64, (q0 % 1024) : (q0 % 1024) + 512
                        ],
                        start=True,
                        stop=True,
                    )
                e_t = epool.tile([128, 1024], BF16, tag="e", name=f"e{b}_{j}")
                if j >= 2 and gi % 3 != 2:
                    nc.scalar.activation(
                        out=e_t[:], in_=st[:], func=AF.Exp, scale=0.125
                    )
                else:
                    nc.vector.tensor_scalar(
                        out=e_t[:].bitcast(U16),
                        in0=st[:],
                        scalar1=EXPA,
                        scalar2=EXPB,
                        op0=ALU.mult,
                        op1=ALU.add,
                    )
                avq.append((j, acc0, acc1, e_t, hp))
                if len(avq) > 2:
                    emit_av(avq.popleft())
                if pend and pend[0][0] <= gi:
                    pend.pop(0)[1]()
                gi += 1
            # schedule normalize for this block (runs during next block)
            ne = make_norm_entries(b, qp, hp, acc0, acc1, outT_sb)
            pend.extend([(gi + d, f) for d, f in ne])
            if hp == 1:
                we = make_wo_entries(qp, outT_sb)
                pend.extend([(gi + d, f) for d, f in we])

        # drain tail
        while avq:
            emit_av(avq.popleft())
        for _, f in pend:
            f()

    nc.compile()
    return nc


def _get_program():
    global _PROGRAM
    if _PROGRAM is None:
        _PROGRAM = build_program()
    return _PROGRAM


def make_in_maps(Q, K, V, mask, W_q, W_k, W_v, W_o):
    bf = ml_dtypes.bfloat16
    Q, K, V = (np.asarray(a, np.float32) for a in (Q, K, V))
    W_q, W_k, W_v, W_o = (np.asarray(a, np.float32) for a in (W_q, W_k, W_v, W_o))
    mask = np.asarray(mask)
    in_maps = []
    for core in range(NCORES):
        b, hg = core // 4, core % 4
        c0 = hg * DL
        in_maps.append(
            {
                "xqT": np.ascontiguousarray(Q[b].T).astype(bf),
                "xkT": np.ascontiguousarray(K[b].T).astype(bf),
                "xvT": np.ascontiguousarray(V[b].T).astype(bf),
                "wq": np.ascontiguousarray(W_q[c0 : c0 + DL, :].T).astype(bf),
                "wk": np.ascontiguousarray(W_k[c0 : c0 + DL, :].T).astype(bf),
                "wv": np.ascontiguousarray(W_v[c0 : c0 + DL, :].T).astype(bf),
                "wo": np.ascontiguousarray(W_o[:, c0 : c0 + DL].T).astype(bf),
                "maskf": np.ascontiguousarray(
                    mask[b].reshape(SEQ // 128, 128).T
                ).astype(np.float32),
            }
        )
    return in_maps


def gather(results):
    out = np.zeros((B, SEQ, D), np.float32)
    for core in range(NCORES):
        out[core // 4] += results[core]["out_part"].astype(np.float32)
    return out


def kernel(Q, K, V, mask, W_q, W_k, W_v, W_o):
    from concourse.bass_utils import run_bass_kernel_spmd

    nc = _get_program()
    in_maps = make_in_maps(Q, K, V, mask, W_q, W_k, W_v, W_o)
    res = run_bass_kernel_spmd(nc, in_maps, list(range(NCORES))).results
    return gather(res)


# revision 36
# speedup vs baseline: 1.1132x; 1.1132x over previous
"""MultiHeadCrossAttention kernel for 8 trn2 NeuronCores.

Reference computation (fp32, per batch b):
    q = Q[b] @ W_q.T ; k = K[b] @ W_k.T ; v = V[b] @ W_v.T      (heads on columns)
    per head h: S = (q_h @ k_h.T) / 8 ; E = exp(S); A = E / E.sum(-1)
    out[b] = concat_h(A @ v_h) @ W_o.T ; rows with mask==0 zeroed

Sharding: 8 cores = (batch b in {0,1}) x (head-group hg in {0..3}, 4 heads each).
Each core computes a partial output  out_part[b] = concat(heads hg) @ W_o[:, cols].T
(stored bf16) and the host sums the 4 partials per batch in f32.

Per-core pipeline (all matmul operands bf16, fp32 PSUM accumulation):
  - Projections as in the earlier design: xT inputs [1024(in), 2048(seq)],
    q/k results stored [128, 2, 2048] with head-pair chunks so the two scores
    matmuls of a pair run on disjoint PE row groups concurrently.
  - Attention inner loop (128 iterations of (qp, hp, j)): scores pair ->
    exp -> PV. The exp of each [128, 1024] scores tile is SPLIT between the
    Scalar engine (exact Exp, columns 0:512 = even head) and the Vector
    engine (Schraudolph bit-trick exp -> uint16 viewed as bf16, columns
    512:1024 = odd head). This halves the per-iteration exp latency and
    roughly doubles exp throughput, which was the baseline bottleneck.
  - PV accumulators acc_h [128, 512] (row 0 = denominator via the ones row
    of vaug; rows 64:128 = out^T). After j=15 the acc is copied to SBUF by
    ScalarE (releases the PSUM bank fast), the denominator row is broadcast
    across partitions via a DRAM bounce, and VectorE computes
    outT = acc / denom with a single tensor_tensor divide (bf16 out).
  - W_o: final[q,:] accumulated over the 256 local dims; the mask (per-q
    0/1) is applied as the per-partition scale of the PSUM->SBUF copy.
  - PE work is software-pipelined: the PV matmuls of iteration i are
    emitted two iterations later so the PE never waits on the exp engines;
    W_o matmuls/copies and the normalize chain are drained from a pending
    queue at a bounded rate between iterations.
"""

import numpy as np
import ml_dtypes
from collections import deque

import concourse.bass as bass
import concourse.bacc as bacc
import concourse.mybir as mybir
import concourse.tile as tile
from contextlib import ExitStack

F32 = mybir.dt.float32
BF16 = mybir.dt.bfloat16
U16 = mybir.dt.uint16
AF = mybir.ActivationFunctionType
ALU = mybir.AluOpType

B = 2
SEQ = 2048          # Sq == Sk
D = 1024            # model dim
DL = 256            # local head dims per core (4 heads x 64)
HL = 4              # local heads
DH = 64             # head dim
NCORES = 8

# Schraudolph exp in bf16 bit space: bits16 = round(s * 0.125 * 128*log2(e)
# + (127*128 + C)).  C = -7.356 centers the ratio approx/exact at 1.0
# (measured on hw: raw trick has mean ratio 1.04064).
EXPA = 0.125 * 128.0 * 1.4426950408889634
EXPB = 16256.0 - 7.3563

_PROGRAM = None


def build_program():
    nc = bacc.Bacc("TRN2", target_bir_lowering=False)

    xqT = nc.declare_dram_parameter("xqT", [D, SEQ], BF16, isOutput=False)
    xkT = nc.declare_dram_parameter("xkT", [D, SEQ], BF16, isOutput=False)
    xvT = nc.declare_dram_parameter("xvT", [D, SEQ], BF16, isOutput=False)
    wq = nc.declare_dram_parameter("wq", [D, DL], BF16, isOutput=False)
    wk = nc.declare_dram_parameter("wk", [D, DL], BF16, isOutput=False)
    wv = nc.declare_dram_parameter("wv", [D, DL], BF16, isOutput=False)
    wo = nc.declare_dram_parameter("wo", [DL, D], BF16, isOutput=False)
    maskf = nc.declare_dram_parameter("maskf", [128, SEQ // 128], F32, isOutput=False)
    out_part = nc.declare_dram_parameter("out_part", [SEQ, D], BF16, isOutput=True)


    with tile.TileContext(nc) as tc, ExitStack() as ctx:
        const = ctx.enter_context(tc.tile_pool(name="const", bufs=1))
        proj = ctx.enter_context(tc.tile_pool(name="proj", bufs=1))
        xpool = ctx.enter_context(tc.tile_pool(name="xpool", bufs=8))
        epool = ctx.enter_context(tc.tile_pool(name="epool", bufs=6))
        opool = ctx.enter_context(tc.tile_pool(name="opool", bufs=2))
        ospool = ctx.enter_context(tc.tile_pool(name="ospool", bufs=3))
        rpool = ctx.enter_context(tc.tile_pool(name="rpool", bufs=4))
        # PSUM: tag st 3x[128,1024] = 6 banks (scores, proj, and wo tiles all
        # rotate this ring), tag acc 2x[128,512] = 2 banks -> 8 total.
        stp = ctx.enter_context(tc.tile_pool(name="stp", bufs=3, space="PSUM"))
        accp = ctx.enter_context(
            tc.tile_pool(name="accp", bufs=2, space="PSUM")
        )
        wop = stp

        # ---------------- constants ----------------
        wq_sb = const.tile([128, 8, DL], BF16)
        wk_sb = const.tile([128, 8, DL], BF16)
        wv_sb = const.tile([128, 8, DL], BF16)
        wo_sb = const.tile([128, 2, D], BF16)
        mask_sb = const.tile([128, SEQ // 128], F32)
        ones_sb = const.tile([1, 128], BF16)
        nc.vector.memset(ones_sb[:], 1.0)
        nc.scalar.dma_start(wq_sb[:], wq[:].rearrange("(a p) d -> p a d", p=128))
        nc.sync.dma_start(wk_sb[:], wk[:].rearrange("(a p) d -> p a d", p=128))
        nc.scalar.dma_start(wv_sb[:], wv[:].rearrange("(a p) d -> p a d", p=128))
        nc.scalar.dma_start(wo_sb[:], wo[:].rearrange("(a p) d -> p a d", p=128))
        nc.scalar.dma_start(mask_sb[:], maskf[:])

        kT0_sb = proj.tile([128, SEQ], BF16)
        kT1_sb = proj.tile([128, SEQ], BF16)
        kTs = (kT0_sb, kT1_sb)
        qTs = [
            [proj.tile([128, 1024], BF16, name=f"qT{dm}_{h}") for h in range(2)]
            for dm in range(2)
        ]
        vaugs = [
            proj.tile([128, HL, 128], BF16, name=f"vaug{j}") for j in range(16)
        ]
        for j in range(16):
            nc.vector.memset(vaugs[j][:], 0.0)
            nc.vector.memset(vaugs[j][:, :, 0:1], 1.0)

        # ---------------- projections ----------------
        for w_sb, xT, which in ((wk_sb, xkT, "k"), (wq_sb, xqT, "q")):
            for half in range(2):
                x_ts = []
                for quarter in range(2):
                    x_t = xpool.tile(
                        [128, 4, 1024], BF16, tag="x", name=f"x{which}{half}{quarter}"
                    )
                    nc.sync.dma_start(
                        x_t[:],
                        xT[
                            quarter * 512 : (quarter + 1) * 512,
                            half * 1024 : (half + 1) * 1024,
                        ].rearrange("(a p) q -> p a q", p=128),
                    )
                    x_ts.append(x_t)
                for dm in range(2):
                    for qc in range(2):
                        ps = stp.tile([128, 512], F32, tag="st", name=f"ps{which}")
                        for ki in range(8):
                            nc.tensor.matmul(
                                ps[:],
                                lhsT=w_sb[:, ki, dm * 128 : (dm + 1) * 128],
                                rhs=x_ts[ki // 4][:, ki % 4, qc * 512 : (qc + 1) * 512],
                                start=(ki == 0),
                                stop=(ki == 7),
                            )
                        col = half * 1024 + qc * 512
                        if which == "k":
                            nc.scalar.copy(out=kTs[dm][:, col : col + 512], in_=ps[:])
                        else:
                            nc.scalar.copy(
                                out=qTs[dm][half][:, qc * 512 : (qc + 1) * 512],
                                in_=ps[:],
                            )
        # v: out v[j*128+p, dl] ; stationary = xvT chunk, moving = wv
        for half in range(2):
            xv_ts = []
            for quarter in range(2):
                x_t = xpool.tile(
                    [128, 4, 1024], BF16, tag="x", name=f"xv{half}{quarter}"
                )
                nc.sync.dma_start(
                    x_t[:],
                    xvT[
                        quarter * 512 : (quarter + 1) * 512,
                        half * 1024 : (half + 1) * 1024,
                    ].rearrange("(a p) q -> p a q", p=128),
                )
                xv_ts.append(x_t)
            for km in range(8):
                ps = stp.tile([128, 512], F32, tag="st", name="psv")
                for ki in range(8):
                    nc.tensor.matmul(
                        ps[:, 0:DL],
                        lhsT=xv_ts[ki // 4][:, ki % 4, km * 128 : (km + 1) * 128],
                        rhs=wv_sb[:, ki, :],
                        start=(ki == 0),
                        stop=(ki == 7),
                    )
                nc.vector.tensor_copy(
                    vaugs[half * 8 + km][:, :, 64 : 64 + DH],
                    ps[:, 0:DL].rearrange("p (h d) -> p h d", h=HL),
                )

        # ---------------- attention + output projection ----------------
        blocks = [(qp, hp) for qp in range(4) for hp in range(2)]
        outTs = {}
        pend = []          # [ready_gi, fn] FIFO
        avq = deque()      # delayed PV emissions (pipeline skew 2)

        def emit_av(entry):
            j, acc0, acc1, e_t, hp_ = entry
            for hi, acc in ((0, acc0), (1, acc1)):
                h = 2 * hp_ + hi
                nc.tensor.matmul(
                    acc[:],
                    lhsT=vaugs[j][:, h, :],
                    rhs=e_t[:, hi * 512 : (hi + 1) * 512],
                    start=(j == 0),
                    stop=(j == 15),
                )

        def make_norm_entries(b, qp, hp, acc0, acc1, outT_sb):
            """Returns [(delay, fn), ...] for the normalize chain of block b.

            recip+copy release the acc bank quickly; the reciprocal row is
            broadcast across partitions with a PE outer product into a
            borrowed st-ring PSUM tile, then one DVE mul (sbuf x psum)."""
            entries = []
            accs = (acc0, acc1)
            acc_sbs = []
            r_sb = rpool.tile([1, 2, 512], BF16, tag="r", name=f"r{b}")
            rb_ps = {}

            def cp(hi):
                acc_sb = rpool.tile(
                    [128, 512], F32, tag="accsb", name=f"accsb{b}_{hi}"
                )
                acc_sbs.append(acc_sb)
                r_f32 = rpool.tile([1, 512], F32, tag="rf", name=f"rf{b}_{hi}")
                nc.vector.reciprocal_approx_fast(out=r_f32[:], in_=accs[hi][0:1, :])
                nc.scalar.copy(
                    out=acc_sb[64:128, :], in_=accs[hi][64:128, :]
                )
                nc.scalar.copy(out=r_sb[:, hi, :], in_=r_f32[:])

            def outer():
                ps = stp.tile([128, 1024], F32, tag="st", name=f"rbps{b}")
                rb_ps[0] = ps
                for hi in range(2):
                    nc.tensor.matmul(
                        ps[:, hi * 512 : (hi + 1) * 512],
                        lhsT=ones_sb[:],
                        rhs=r_sb[:, hi, :],
                        start=True,
                        stop=True,
                    )

            def mul(hi):
                nc.vector.tensor_mul(
                    outT_sb[hi * 64 : (hi + 1) * 64, hp, :],
                    acc_sbs[hi][64 : 64 + DH, :],
                    rb_ps[0][64:128, hi * 512 : (hi + 1) * 512],
                )

            entries.append((1, lambda: cp(0)))
            entries.append((2, lambda: cp(1)))
            entries.append((6, outer))
            entries.append((8, lambda: mul(0)))
            entries.append((9, lambda: mul(1)))
            return entries

        wo_cp_ctr = [0]

        def make_wo_entries(qp, outT_sb):
            entries = []
            for mq in range(4):
                o_sb = ospool.tile([128, 1024], BF16, tag="o", name=f"o{qp}_{mq}")
                qg = qp * 4 + mq
                ps_ref = {}

                def mk_mm(oc, mq=mq, o_sb=o_sb, ps_ref=ps_ref):
                    def f():
                        ps = wop.tile(
                            [128, 512], F32, tag="st", name=f"wops{qp}_{mq}_{oc}"
                        )
                        ps_ref[oc] = ps
                        for kc in range(2):
                            nc.tensor.matmul(
                                ps[:],
                                lhsT=outT_sb[:, kc, mq * 128 : (mq + 1) * 128],
                                rhs=wo_sb[:, kc, oc * 512 : (oc + 1) * 512],
                                start=(kc == 0),
                                stop=(kc == 1),
                            )
                    return f

                def mk_cp(oc, last, mq=mq, o_sb=o_sb, ps_ref=ps_ref, qg=qg):
                    def f():
                        i = wo_cp_ctr[0]
                        wo_cp_ctr[0] += 1
                        if i % 2 == 0:
                            nc.scalar.activation(
                                out=o_sb[:, oc * 512 : (oc + 1) * 512],
                                in_=ps_ref[oc][:],
                                func=AF.Copy,
                                scale=mask_sb[:, qg : qg + 1],
                            )
                        else:
                            nc.vector.tensor_scalar_mul(
                                o_sb[:, oc * 512 : (oc + 1) * 512],
                                ps_ref[oc][:],
                                mask_sb[:, qg : qg + 1],
                            )
                        if last:
                            nc.sync.dma_start(
                                out_part[qg * 128 : (qg + 1) * 128, :], o_sb[:]
                            )
                    return f

                base = 9 + mq * 6
                entries.append((base, mk_mm(0)))
                entries.append((base + 2, lambda f1=mk_cp(0, False), f2=mk_mm(1): (f1(), f2())))
                entries.append((base + 4, mk_cp(1, True)))
            return entries

        gi = 0
        for b, (qp, hp) in enumerate(blocks):
            acc0 = accp.tile([128, 512], F32, tag="acc", name=f"acc{b}_0")
            acc1 = accp.tile([128, 512], F32, tag="acc", name=f"acc{b}_1")
            if qp not in outTs:
                outTs[qp] = opool.tile(
                    [128, 2, 512], BF16, tag="outT", name=f"outT{qp}"
                )
            outT_sb = outTs[qp]
            q0 = qp * 512
            for j in range(16):
                st = stp.tile([128, 1024], F32, tag="st", name=f"st{b}_{j}")
                for hi in range(2):
                    r0 = hi * 64
                    nc.tensor.matmul(
                        st[:, hi * 512 : (hi + 1) * 512],
                        lhsT=kTs[hp][r0 : r0 + 64, j * 128 : (j + 1) * 128],
                        rhs=qTs[hp][q0 // 1024][
                            r0 : r0 + 64, (q0 % 1024) : (q0 % 1024) + 512
                        ],
                        start=True,
                        stop=True,
                    )
                e_t = epool.tile([128, 1024], BF16, tag="e", name=f"e{b}_{j}")
                if gi % 3 != 2:
                    nc.scalar.activation(
                        out=e_t[:], in_=st[:], func=AF.Exp, scale=0.125
                    )
                else:
                    nc.vector.tensor_scalar(
                        out=e_t[:].bitcast(U16),
                        in0=st[:],
                        scalar1=EXPA,
                        scalar2=EXPB,
                        op0=ALU.mult,
                        op1=ALU.add,
                    )
                avq.append((j, acc0, acc1, e_t, hp))
                if len(avq) > 2:
                    emit_av(avq.popleft())
                if pend and pend[0][0] <= gi:
                    pend.pop(0)[1]()
                gi += 1
            # schedule normalize for this block (runs during next block)
            ne = make_norm_entries(b, qp, hp, acc0, acc1, outT_sb)
            pend.extend([(gi + d, f) for d, f in ne])
            if hp == 1:
                we = make_wo_entries(qp, outT_sb)
                pend.extend([(gi + d, f) for d, f in we])

        # drain tail
        while avq:
            emit_av(avq.popleft())
        for _, f in pend:
            f()

    nc.compile()
    return nc


def _get_program():
    global _PROGRAM
    if _PROGRAM is None:
        _PROGRAM = build_program()
    return _PROGRAM


def make_in_maps(Q, K, V, mask, W_q, W_k, W_v, W_o):
    bf = ml_dtypes.bfloat16
    Q, K, V = (np.asarray(a, np.float32) for a in (Q, K, V))
    W_q, W_k, W_v, W_o = (np.asarray(a, np.float32) for a in (W_q, W_k, W_v, W_o))
    mask = np.asarray(mask)
    in_maps = []
    for core in range(NCORES):
        b, hg = core // 4, core % 4
        c0 = hg * DL
        in_maps.append(
            {
                "xqT": np.ascontiguousarray(Q[b].T).astype(bf),
                "xkT": np.ascontiguousarray(K[b].T).astype(bf),
                "xvT": np.ascontiguousarray(V[b].T).astype(bf),
                "wq": np.ascontiguousarray(W_q[c0 : c0 + DL, :].T).astype(bf),
                "wk": np.ascontiguousarray(W_k[c0 : c0 + DL, :].T).astype(bf),
                "wv": np.ascontiguousarray(W_v[c0 : c0 + DL, :].T).astype(bf),
                "wo": np.ascontiguousarray(W_o[:, c0 : c0 + DL].T).astype(bf),
                "maskf": np.ascontiguousarray(
                    mask[b].reshape(SEQ // 128, 128).T
                ).astype(np.float32),
            }
        )
    return in_maps


def gather(results):
    out = np.zeros((B, SEQ, D), np.float32)
    for core in range(NCORES):
        out[core // 4] += results[core]["out_part"].astype(np.float32)
    return out


def kernel(Q, K, V, mask, W_q, W_k, W_v, W_o):
    from concourse.bass_utils import run_bass_kernel_spmd

    nc = _get_program()
    in_maps = make_in_maps(Q, K, V, mask, W_q, W_k, W_v, W_o)
    res = run_bass_kernel_spmd(nc, in_maps, list(range(NCORES))).results
    return gather(res)
